# revision 20
# baseline (speedup 1.0000x reference)
"""Causal self-attention Trainium2 kernel (8 NeuronCores).

Sharding (Megatron-style, per sharding_hint):
  core c -> batch b = c//2, head-group g = c%2 (8 of 16 heads).
  W_q/W_k/W_v column-sliced per head group; W_o row-sliced; host sums the
  two partial outputs per batch (tensor-parallel reduce) and adds b_o.

Per-core kernel (all matmuls bf16 with fp32 PSUM accumulation):
  xT    [1024, 2048]  x[b] transposed (d_emb on partitions)
  wqkv  [1024, 1536]  [Wq_g | Wk_g | Wv_g]
  wo    [512, 1024]   W_o rows for this head group
  out   [2048, 1024]  fp32 partial (no bias)

Layouts: qT/kT stored [head_dim, n] so score matmuls contract over the
64-dim head axis; the two heads of a partition-tile occupy partitions
0:64 / 64:128, and their score matmuls are emitted interleaved so the PE
runs them concurrently in different row groups. Scores are computed
TRANSPOSED ([k, q]) so the exp'd weights feed the ctx matmul directly as
the moving operand; v is kept [n, head_dim] with a ones-block per head so
a single matmul yields both ctx^T and the softmax denominators broadcast
across 64 partitions.

Pipelining: QKV projection work is split into per-chunk units (v for the
chunk's 4 n-tiles, q/k for the chunk's 512 q-columns) and the units for
chunk qc+1 are emitted interleaved INSIDE attention chunk qc, so the PE
fills exp-latency gaps with projection matmuls and the scalar engine
starts exp'ing ~70us earlier than a proj-then-attend ordering. All PSUM
score/projection/out-proj tiles share one 2-slot pool ([128,1024], 4
banks) and the two ctx accumulators use the other 4 banks.

Causal handling: k-tiles strictly above the diagonal are skipped; on
diagonal sub-tiles the fully-masked query prefix is never computed
(scores and ctx matmuls trim their moving operand to q >= k-block start,
and exp starts at the first live column), and only the 128x128 diagonal
block gets a triangular bf16 multiply.

`reps` repeats the whole body inside one NEFF — used only for timing
((T(n)-T(1))/(n-1) cancels dispatch overhead); the graded path is reps=1.
"""

import sys

import numpy as np

sys.path.insert(0, "/opt/trn_rl_repo")

import ml_dtypes

BF16 = ml_dtypes.bfloat16

D_EMB = 1024
N_SEQ = 2048
N_HEADS_CORE = 8  # heads per core
HD = 64  # head dim
KT = D_EMB // 128  # 8 k-tiles over d_emb
PT = 4  # partition tiles over the 512 per-core head dims
NT = N_SEQ // 128  # 16 n-tiles
QC = N_SEQ // 512  # 4 query chunks of 512
SCALE = 1.0 / np.sqrt(HD)

_CACHE = {}


def _emit_body(nc, tc, mybir, sfx, xT_d, wqkv_d, wo_d, out_d):
    f32 = mybir.dt.float32
    bf16 = mybir.dt.bfloat16

    with tc.tile_pool(name=f"persist{sfx}", bufs=1) as persist:
        wo_sb = [
            persist.tile([128, D_EMB], bf16, name=f"wo{p}{sfx}", tag=f"wo{p}")
            for p in range(PT)
        ]
        qt_sb = [
            persist.tile([128, N_SEQ], bf16, name=f"qt{p}{sfx}", tag=f"qt{p}")
            for p in range(PT)
        ]
        kt_sb = [
            persist.tile([128, N_SEQ], bf16, name=f"kt{p}{sfx}", tag=f"kt{p}")
            for p in range(PT)
        ]
        ctxt_sb = [
            persist.tile([128, N_SEQ], bf16, name=f"ctxt{p}{sfx}", tag=f"ctxt{p}")
            for p in range(PT)
        ]
        # v per n-tile [128, 1024]: head h -> cols h*128:h*128+64 = v_h,
        # cols h*128+64:h*128+128 = 1.0 (softmax denominator ones-trick)
        v_sb = [
            persist.tile([128, 1024], bf16, name=f"v{nt}{sfx}", tag=f"v{nt}")
            for nt in range(NT)
        ]
        tri_sb = persist.tile([128, 128], bf16, name=f"tri{sfx}", tag="tri")
        xt_sb = [
            persist.tile([128, N_SEQ], bf16, name=f"xt{k}{sfx}", tag=f"xt{k}")
            for k in range(KT)
        ]
        wqkv_sb = [
            persist.tile([128, 1536], bf16, name=f"wqkv{k}{sfx}", tag=f"wqkv{k}")
            for k in range(KT)
        ]

        def vaug_ap(nt, h):
            return v_sb[nt][:, h * 128 : (h + 1) * 128]

        # ---- constants (gpsimd, no deps) ----
        for nt in range(NT):
            ones_view = v_sb[nt].rearrange("p (h c) -> p h c", h=N_HEADS_CORE)
            nc.gpsimd.memset(ones_view[:, :, 64:128], 1.0)
        # tri[k_local, q_local] = 1.0 if q_local >= k_local else 0
        nc.gpsimd.memset(tri_sb[:], 1.0)
        nc.gpsimd.affine_select(
            out=tri_sb[:],
            in_=tri_sb[:],
            compare_op=mybir.AluOpType.is_ge,
            fill=0.0,
            base=0,
            pattern=[[1, 128]],
            channel_multiplier=-1,
        )

        # ---- input DMA, balanced across the three DMA-capable queues ----
        for k in range(KT):
            xq = nc.sync if k % 2 == 0 else nc.scalar
            xq.dma_start(
                out=xt_sb[k][:], in_=xT_d[k * 128 : (k + 1) * 128, :]
            )
            wq = nc.gpsimd if k < 6 else nc.scalar
            wq.dma_start(
                out=wqkv_sb[k][:], in_=wqkv_d[k * 128 : (k + 1) * 128, :]
            )
        for p in range(PT):
            # wo is first needed by out(0) units draining in chunk 1 (~60us)
            nc.sync.dma_start(
                out=wo_sb[p][:], in_=wo_d[p * 128 : (p + 1) * 128, :]
            )

        with (
            tc.tile_pool(name=f"expp{sfx}", bufs=4) as expp,
            tc.tile_pool(name=f"rpool{sfx}", bufs=2) as rpool,
            tc.tile_pool(name=f"outp{sfx}", bufs=3) as outp,
            tc.tile_pool(name=f"pssc{sfx}", bufs=2, space="PSUM") as pssc_pool,
            tc.tile_pool(name=f"psctx{sfx}", bufs=1, space="PSUM") as psctx_pool,
            tc.tile_pool(name=f"psaux{sfx}", bufs=2, space="PSUM") as psaux_pool,
        ):
            # background work units, drained interleaved inside the
            # attention group loop. Drained units use the 2-slot 1-bank aux
            # PSUM ring so they never block on the score ring; fill-phase
            # units also borrow the score ring, idle before attention.
            def unit_v(nt, pool, tag):
                def emit():
                    ps = pool.tile(
                        [128, 512], f32, name=f"psv{nt}{sfx}", tag=tag
                    )
                    for k in range(KT):
                        nc.tensor.matmul(
                            ps[:],
                            lhsT=xt_sb[k][:, nt * 128 : (nt + 1) * 128],
                            rhs=wqkv_sb[k][:, 1024:1536],
                            start=(k == 0),
                            stop=(k == KT - 1),
                        )
                    v_view = v_sb[nt].rearrange("p (h c) -> p h c", h=N_HEADS_CORE)
                    nc.vector.tensor_copy(
                        v_view[:, :, 0:64],
                        ps.rearrange("p (h c) -> p h c", h=N_HEADS_CORE),
                    )

                return emit

            def unit_qk(which, p, qn, pool, tag):
                # which: 0 -> q, 1 -> k, for chunk qn, partition-tile p
                def emit():
                    nsl = slice(qn * 512, (qn + 1) * 512)
                    dst = (qt_sb, kt_sb)[which]
                    ps = pool.tile(
                        [128, 512], f32, name=f"psqk{which}_{p}_{qn}{sfx}",
                        tag=tag,
                    )
                    col0 = which * 512 + p * 128
                    for k in range(KT):
                        nc.tensor.matmul(
                            ps[:],
                            lhsT=wqkv_sb[k][:, col0 : col0 + 128],
                            rhs=xt_sb[k][:, nsl],
                            start=(k == 0),
                            stop=(k == KT - 1),
                        )
                    nc.vector.tensor_copy(dst[p][:, nsl], ps[:])

                return emit

            def unit_out(nt, pool, tag):
                # out = ctx @ Wo for one n-tile, in two 512-wide halves so
                # each copy+DMA drains while the other half accumulates
                def emit():
                    osb = outp.tile(
                        [128, 1024], f32, name=f"osb{nt}{sfx}", tag="osb"
                    )
                    for dh in range(2):
                        pso = pool.tile(
                            [128, 512], f32, name=f"pso{nt}_{dh}{sfx}",
                            tag=tag,
                        )
                        for p in range(PT):
                            nc.tensor.matmul(
                                pso[:],
                                lhsT=ctxt_sb[p][:, nt * 128 : (nt + 1) * 128],
                                rhs=wo_sb[p][:, dh * 512 : (dh + 1) * 512],
                                start=(p == 0),
                                stop=(p == PT - 1),
                            )
                        nc.vector.tensor_copy(
                            osb[:, dh * 512 : (dh + 1) * 512], pso[:]
                        )
                        nc.sync.dma_start(
                            out=out_d[
                                nt * 128 : (nt + 1) * 128,
                                dh * 512 : (dh + 1) * 512,
                            ],
                            in_=osb[:, dh * 512 : (dh + 1) * 512],
                        )

                return emit

            def vq(nt):
                return unit_v(nt, psaux_pool, "aux")

            def qkq(qn):
                us = []
                for p in range(PT):
                    us.append(unit_qk(0, p, qn, psaux_pool, "aux"))
                    us.append(unit_qk(1, p, qn, psaux_pool, "aux"))
                return us

            # fill phase: chunk 0's q/k/v, on the idle score+aux rings
            fill = []
            for p in range(PT):
                fill.append(unit_qk(0, p, 0, pssc_pool, "sc"))
                fill.append(unit_qk(1, p, 0, psaux_pool, "aux"))
            fill += [unit_v(nt, pssc_pool, "sc") for nt in range(4)]
            for u in fill:
                u()

            # per-chunk drain lists (deps: chunk qc's q/k/v units must drain
            # by chunk qc-1's end; out(c) drains any time after chunk c)
            drain = {
                0: qkq(1) + [vq(nt) for nt in range(4, 8)],
                1: qkq(2) + [vq(nt) for nt in range(8, 12)]
                + [unit_out(nt, psaux_pool, "aux") for nt in range(0, 4)],
                2: qkq(3) + [vq(nt) for nt in range(12, 16)]
                + [unit_out(nt, psaux_pool, "aux") for nt in range(4, 8)],
                3: [unit_out(nt, psaux_pool, "aux") for nt in range(8, 12)],
            }

            for qc in range(QC):
                q0 = qc * 512
                nk = 4 * qc + 4  # causal: k-tiles 0..nk-1
                ngroups = nk // 2
                units = drain[qc]
                nslots = PT * ngroups
                sched = [[] for _ in range(nslots + 1)]
                for i in range(len(units)):
                    sched[round((i + 1) * nslots / len(units))].append(i)
                slot = 0
                for p in range(PT):
                    # both heads' ctx in one 2-bank tile: h2 -> cols h2*512
                    ctx_ps = psctx_pool.tile(
                        [128, 1024], f32, name=f"ctx{p}_{qc}{sfx}", tag="ctx"
                    )
                    pend = None  # deferred ctx-matmul emission (1-group lag)
                    for gi in range(ngroups):
                        ps = [
                            pssc_pool.tile(
                                [128, 1024],
                                f32,
                                name=f"sc{p}_{qc}_{gi}_{h2}{sfx}",
                                tag="sc",
                            )
                            for h2 in range(2)
                        ]
                        ex = [
                            expp.tile(
                                [128, 1024],
                                bf16,
                                name=f"ex{p}_{qc}_{gi}_{h2}{sfx}",
                                tag="ex",
                            )
                            for h2 in range(2)
                        ]
                        # interleave heads so PE overlaps the row-group pairs
                        for j in range(2):
                            ki = 2 * gi + j
                            jj = ki - 4 * qc  # >=0 on diagonal sub-tiles
                            t0 = max(0, 128 * jj)  # masked-prefix trim
                            for h2 in range(2):
                                hb = h2 * 64
                                nc.tensor.matmul(
                                    ps[h2][:, j * 512 + t0 : (j + 1) * 512],
                                    lhsT=kt_sb[p][
                                        hb : hb + 64, ki * 128 : (ki + 1) * 128
                                    ],
                                    rhs=qt_sb[p][hb : hb + 64, q0 + t0 : q0 + 512],
                                    start=True,
                                    stop=True,
                                )
                        # exp from the first live column onward (cols below
                        # are never read by the ctx matmuls)
                        e0 = max(0, 128 * (2 * gi - 4 * qc))
                        for h2 in range(2):
                            nc.scalar.activation(
                                ex[h2][:, e0:1024],
                                ps[h2][:, e0:1024],
                                mybir.ActivationFunctionType.Exp,
                                scale=float(SCALE),
                            )
                        for j in range(2):
                            ki = 2 * gi + j
                            jj = ki - 4 * qc
                            if jj >= 0:  # triangular block on the diagonal
                                blk = slice(
                                    j * 512 + 128 * jj, j * 512 + 128 * jj + 128
                                )
                                for h2 in range(2):
                                    nc.vector.tensor_mul(
                                        ex[h2][:, blk], ex[h2][:, blk], tri_sb[:]
                                    )
                        if pend is not None:
                            pend()

                        def pend(gi=gi, ex=ex):
                            for j in range(2):
                                ki = 2 * gi + j
                                jj = ki - 4 * qc
                                t0 = max(0, 128 * jj)
                                for h2 in range(2):
                                    h = 2 * p + h2
                                    nc.tensor.matmul(
                                        ctx_ps[:, h2 * 512 + t0 : (h2 + 1) * 512],
                                        lhsT=vaug_ap(ki, h),
                                        rhs=ex[h2][:, j * 512 + t0 : (j + 1) * 512],
                                        start=(ki == 0),
                                        stop=(ki == nk - 1),
                                    )

                        slot += 1
                        for i in sched[slot]:
                            units[i]()
                    pend()
                    rec = rpool.tile(
                        [64, 1024], f32, name=f"rec{p}_{qc}{sfx}", tag="rec"
                    )
                    nc.vector.reciprocal(rec[:], ctx_ps[64:128, :])
                    # on the final chunk, emit the normalizing muls in
                    # 128-col pieces so the tail out-projection's first
                    # n-tiles unblock before the whole 512-col mul finishes
                    pieces = 4 if qc == QC - 1 else 1
                    w = 512 // pieces
                    for pc in range(pieces):
                        for h2 in range(2):
                            c0 = h2 * 512 + pc * w
                            nc.vector.tensor_mul(
                                ctxt_sb[p][
                                    h2 * 64 : h2 * 64 + 64,
                                    q0 + pc * w : q0 + (pc + 1) * w,
                                ],
                                ctx_ps[0:64, c0 : c0 + w],
                                rec[:, c0 : c0 + w],
                            )

            # tail: the last chunk's out-projection, alternating the freed
            # score ring with aux for a deeper pipeline
            for i, nt in enumerate(range(N_SEQ // 128 - 4, N_SEQ // 128)):
                if i % 2 == 0:
                    unit_out(nt, pssc_pool, "sc")()
                else:
                    unit_out(nt, psaux_pool, "aux")()


def _build_module(reps=1):
    import concourse.bacc as bacc
    import concourse.mybir as mybir
    import concourse.tile as tile

    f32 = mybir.dt.float32
    bf16 = mybir.dt.bfloat16

    nc = bacc.Bacc()
    xT_d = nc.dram_tensor("xT", [D_EMB, N_SEQ], bf16, kind="ExternalInput")
    wqkv_d = nc.dram_tensor("wqkv", [D_EMB, 1536], bf16, kind="ExternalInput")
    wo_d = nc.dram_tensor("wo", [512, D_EMB], bf16, kind="ExternalInput")
    out_d = nc.dram_tensor("out", [N_SEQ, D_EMB], f32, kind="ExternalOutput")

    with tile.TileContext(nc) as tc:
        for rep in range(reps):
            _emit_body(
                nc, tc, mybir, f"_r{rep}" if reps > 1 else "",
                xT_d, wqkv_d, wo_d, out_d,
            )

    if not nc.is_finalized():
        nc.finalize()
    return nc


def _get_module(reps=1):
    key = f"nc{reps}"
    if key not in _CACHE:
        _CACHE[key] = _build_module(reps)
    return _CACHE[key]


def make_in_maps(x, W_q, W_k, W_v, W_o):
    in_maps = []
    for c in range(8):
        b, g = c // 2, c % 2
        gs = slice(g * 512, (g + 1) * 512)
        xT = np.ascontiguousarray(x[b].T).astype(BF16)
        wqkv = np.concatenate(
            [W_q[:, gs], W_k[:, gs], W_v[:, gs]], axis=1
        ).astype(BF16)
        wo = np.ascontiguousarray(W_o[gs, :]).astype(BF16)
        in_maps.append({"xT": xT, "wqkv": wqkv, "wo": wo})
    return in_maps


def kernel(x, W_q, W_k, W_v, W_o, b_o):
    from concourse.bass_utils import run_bass_kernel_spmd

    nc = _get_module()
    in_maps = make_in_maps(x, W_q, W_k, W_v, W_o)
    res = run_bass_kernel_spmd(nc, in_maps, core_ids=list(range(8)))

    out = np.empty((4, N_SEQ, D_EMB), np.float32)
    for b in range(4):
        out[b] = (
            res.results[2 * b]["out"]
            + res.results[2 * b + 1]["out"]
            + b_o[None, :].astype(np.float32)
        )
    return out


# revision 21
# speedup vs baseline: 1.0163x; 1.0163x over previous
"""Causal self-attention Trainium2 kernel (8 NeuronCores).

Sharding (Megatron-style, per sharding_hint):
  core c -> batch b = c//2, head-group g = c%2 (8 of 16 heads).
  W_q/W_k/W_v column-sliced per head group; W_o row-sliced; host sums the
  two partial outputs per batch (tensor-parallel reduce) and adds b_o.

Per-core kernel (all matmuls bf16 with fp32 PSUM accumulation):
  xT    [1024, 2048]  x[b] transposed (d_emb on partitions)
  wqkv  [1024, 1536]  [Wq_g | Wk_g | Wv_g]
  wo    [512, 1024]   W_o rows for this head group
  out   [2048, 1024]  fp32 partial (no bias)

Layouts: qT/kT stored [head_dim, n] so score matmuls contract over the
64-dim head axis; the two heads of a partition-tile occupy partitions
0:64 / 64:128, and their score matmuls are emitted interleaved so the PE
runs them concurrently in different row groups. Scores are computed
TRANSPOSED ([k, q]) so the exp'd weights feed the ctx matmul directly as
the moving operand; v is kept [n, head_dim] with a ones-block per head so
a single matmul yields both ctx^T and the softmax denominators broadcast
across 64 partitions.

Pipelining: QKV projection work is split into per-chunk units (v for the
chunk's 4 n-tiles, q/k for the chunk's 512 q-columns) and the units for
chunk qc+1 are emitted interleaved INSIDE attention chunk qc, so the PE
fills exp-latency gaps with projection matmuls and the scalar engine
starts exp'ing ~70us earlier than a proj-then-attend ordering. All PSUM
score/projection/out-proj tiles share one 2-slot pool ([128,1024], 4
banks) and the two ctx accumulators use the other 4 banks.

Causal handling: k-tiles strictly above the diagonal are skipped; on
diagonal sub-tiles the fully-masked query prefix is never computed
(scores and ctx matmuls trim their moving operand to q >= k-block start,
and exp starts at the first live column), and only the 128x128 diagonal
block gets a triangular bf16 multiply.

`reps` repeats the whole body inside one NEFF — used only for timing
((T(n)-T(1))/(n-1) cancels dispatch overhead); the graded path is reps=1.
"""

import sys

import numpy as np

sys.path.insert(0, "/opt/trn_rl_repo")

import ml_dtypes

BF16 = ml_dtypes.bfloat16

D_EMB = 1024
N_SEQ = 2048
N_HEADS_CORE = 8  # heads per core
HD = 64  # head dim
KT = D_EMB // 128  # 8 k-tiles over d_emb
PT = 4  # partition tiles over the 512 per-core head dims
NT = N_SEQ // 128  # 16 n-tiles
QC = N_SEQ // 512  # 4 query chunks of 512
SCALE = 1.0 / np.sqrt(HD)

_CACHE = {}


def _emit_body(nc, tc, mybir, sfx, xT_d, wqkv_d, wo_d, out_d):
    f32 = mybir.dt.float32
    bf16 = mybir.dt.bfloat16

    with tc.tile_pool(name=f"persist{sfx}", bufs=1) as persist:
        wo_sb = [
            persist.tile([128, D_EMB], bf16, name=f"wo{p}{sfx}", tag=f"wo{p}")
            for p in range(PT)
        ]
        qt_sb = [
            persist.tile([128, N_SEQ], bf16, name=f"qt{p}{sfx}", tag=f"qt{p}")
            for p in range(PT)
        ]
        kt_sb = [
            persist.tile([128, N_SEQ], bf16, name=f"kt{p}{sfx}", tag=f"kt{p}")
            for p in range(PT)
        ]
        ctxt_sb = [
            persist.tile([128, N_SEQ], bf16, name=f"ctxt{p}{sfx}", tag=f"ctxt{p}")
            for p in range(PT)
        ]
        # v per n-tile [128, 1024]: head h -> cols h*128:h*128+64 = v_h,
        # cols h*128+64:h*128+128 = 1.0 (softmax denominator ones-trick)
        v_sb = [
            persist.tile([128, 1024], bf16, name=f"v{nt}{sfx}", tag=f"v{nt}")
            for nt in range(NT)
        ]
        tri_sb = persist.tile([128, 128], bf16, name=f"tri{sfx}", tag="tri")
        xt_sb = [
            persist.tile([128, N_SEQ], bf16, name=f"xt{k}{sfx}", tag=f"xt{k}")
            for k in range(KT)
        ]
        wqkv_sb = [
            persist.tile([128, 1536], bf16, name=f"wqkv{k}{sfx}", tag=f"wqkv{k}")
            for k in range(KT)
        ]

        def vaug_ap(nt, h):
            return v_sb[nt][:, h * 128 : (h + 1) * 128]

        # ---- constants (gpsimd, no deps) ----
        for nt in range(NT):
            ones_view = v_sb[nt].rearrange("p (h c) -> p h c", h=N_HEADS_CORE)
            nc.gpsimd.memset(ones_view[:, :, 64:128], 1.0)
        # tri[k_local, q_local] = 1.0 if q_local >= k_local else 0
        nc.gpsimd.memset(tri_sb[:], 1.0)
        nc.gpsimd.affine_select(
            out=tri_sb[:],
            in_=tri_sb[:],
            compare_op=mybir.AluOpType.is_ge,
            fill=0.0,
            base=0,
            pattern=[[1, 128]],
            channel_multiplier=-1,
        )

        # ---- input DMA, balanced across the three DMA-capable queues ----
        for k in range(KT):
            xq = nc.sync if k % 2 == 0 else nc.scalar
            xq.dma_start(
                out=xt_sb[k][:], in_=xT_d[k * 128 : (k + 1) * 128, :]
            )
            wq = nc.gpsimd if k < 6 else nc.scalar
            wq.dma_start(
                out=wqkv_sb[k][:], in_=wqkv_d[k * 128 : (k + 1) * 128, :]
            )
        for p in range(PT):
            # wo is first needed by out(0) units draining in chunk 1 (~60us)
            nc.sync.dma_start(
                out=wo_sb[p][:], in_=wo_d[p * 128 : (p + 1) * 128, :]
            )

        with (
            tc.tile_pool(name=f"expp{sfx}", bufs=4) as expp,
            tc.tile_pool(name=f"rpool{sfx}", bufs=2) as rpool,
            tc.tile_pool(name=f"outp{sfx}", bufs=3) as outp,
            tc.tile_pool(name=f"pssc{sfx}", bufs=2, space="PSUM") as pssc_pool,
            tc.tile_pool(name=f"psctx{sfx}", bufs=1, space="PSUM") as psctx_pool,
            tc.tile_pool(name=f"psaux{sfx}", bufs=2, space="PSUM") as psaux_pool,
        ):
            # background work units, drained interleaved inside the
            # attention group loop. Drained units use the 2-slot 1-bank aux
            # PSUM ring so they never block on the score ring; fill-phase
            # units also borrow the score ring, idle before attention.
            def unit_v(nt, pool, tag):
                def emit():
                    ps = pool.tile(
                        [128, 512], f32, name=f"psv{nt}{sfx}", tag=tag
                    )
                    for k in range(KT):
                        nc.tensor.matmul(
                            ps[:],
                            lhsT=xt_sb[k][:, nt * 128 : (nt + 1) * 128],
                            rhs=wqkv_sb[k][:, 1024:1536],
                            start=(k == 0),
                            stop=(k == KT - 1),
                        )
                    v_view = v_sb[nt].rearrange("p (h c) -> p h c", h=N_HEADS_CORE)
                    nc.vector.tensor_copy(
                        v_view[:, :, 0:64],
                        ps.rearrange("p (h c) -> p h c", h=N_HEADS_CORE),
                    )

                return emit

            def unit_qk(which, p, qn, pool, tag):
                # which: 0 -> q, 1 -> k, for chunk qn, partition-tile p
                def emit():
                    nsl = slice(qn * 512, (qn + 1) * 512)
                    dst = (qt_sb, kt_sb)[which]
                    ps = pool.tile(
                        [128, 512], f32, name=f"psqk{which}_{p}_{qn}{sfx}",
                        tag=tag,
                    )
                    col0 = which * 512 + p * 128
                    for k in range(KT):
                        nc.tensor.matmul(
                            ps[:],
                            lhsT=wqkv_sb[k][:, col0 : col0 + 128],
                            rhs=xt_sb[k][:, nsl],
                            start=(k == 0),
                            stop=(k == KT - 1),
                        )
                    nc.vector.tensor_copy(dst[p][:, nsl], ps[:])

                return emit

            def unit_out(nt, pool, tag):
                # out = ctx @ Wo for one n-tile, in two 512-wide halves so
                # each copy+DMA drains while the other half accumulates
                def emit():
                    osb = outp.tile(
                        [128, 1024], f32, name=f"osb{nt}{sfx}", tag="osb"
                    )
                    for dh in range(2):
                        pso = pool.tile(
                            [128, 512], f32, name=f"pso{nt}_{dh}{sfx}",
                            tag=tag,
                        )
                        for p in range(PT):
                            nc.tensor.matmul(
                                pso[:],
                                lhsT=ctxt_sb[p][:, nt * 128 : (nt + 1) * 128],
                                rhs=wo_sb[p][:, dh * 512 : (dh + 1) * 512],
                                start=(p == 0),
                                stop=(p == PT - 1),
                            )
                        nc.vector.tensor_copy(
                            osb[:, dh * 512 : (dh + 1) * 512], pso[:]
                        )
                        nc.sync.dma_start(
                            out=out_d[
                                nt * 128 : (nt + 1) * 128,
                                dh * 512 : (dh + 1) * 512,
                            ],
                            in_=osb[:, dh * 512 : (dh + 1) * 512],
                        )

                return emit

            def vq(nt):
                return unit_v(nt, psaux_pool, "aux")

            def qkq(qn):
                us = []
                for p in range(PT):
                    us.append(unit_qk(0, p, qn, psaux_pool, "aux"))
                    us.append(unit_qk(1, p, qn, psaux_pool, "aux"))
                return us

            # fill phase: chunk 0's q/k/v, on the idle score+aux rings
            fill = []
            for p in range(PT):
                fill.append(unit_qk(0, p, 0, pssc_pool, "sc"))
                fill.append(unit_qk(1, p, 0, psaux_pool, "aux"))
            fill += [unit_v(nt, pssc_pool, "sc") for nt in range(4)]
            for u in fill:
                u()

            # per-chunk drain lists (deps: chunk qc's q/k/v units must drain
            # by chunk qc-1's end; out(c) drains any time after chunk c)
            drain = {
                0: qkq(1) + [vq(nt) for nt in range(4, 8)],
                1: qkq(2) + [vq(nt) for nt in range(8, 12)]
                + [unit_out(nt, psaux_pool, "aux") for nt in range(0, 4)],
                2: qkq(3) + [vq(nt) for nt in range(12, 16)]
                + [unit_out(nt, psaux_pool, "aux") for nt in range(4, 8)],
                3: [unit_out(nt, psaux_pool, "aux") for nt in range(8, 12)],
            }

            for qc in range(QC):
                q0 = qc * 512
                nk = 4 * qc + 4  # causal: k-tiles 0..nk-1
                ngroups = nk // 2
                units = drain[qc]
                nslots = PT * ngroups
                sched = [[] for _ in range(nslots + 1)]
                for i in range(len(units)):
                    sched[round((i + 1) * nslots / len(units))].append(i)
                slot = 0
                for p in range(PT):
                    # both heads' ctx in one 2-bank tile: h2 -> cols h2*512
                    ctx_ps = psctx_pool.tile(
                        [128, 1024], f32, name=f"ctx{p}_{qc}{sfx}", tag="ctx"
                    )
                    pend = None  # deferred ctx-matmul emission (1-group lag)
                    for gi in range(ngroups):
                        ps = [
                            pssc_pool.tile(
                                [128, 1024],
                                f32,
                                name=f"sc{p}_{qc}_{gi}_{h2}{sfx}",
                                tag="sc",
                            )
                            for h2 in range(2)
                        ]
                        ex = [
                            expp.tile(
                                [128, 1024],
                                bf16,
                                name=f"ex{p}_{qc}_{gi}_{h2}{sfx}",
                                tag="ex",
                            )
                            for h2 in range(2)
                        ]
                        # interleave heads so PE overlaps the row-group pairs
                        for j in range(2):
                            ki = 2 * gi + j
                            jj = ki - 4 * qc  # >=0 on diagonal sub-tiles
                            t0 = max(0, 128 * jj)  # masked-prefix trim
                            for h2 in range(2):
                                hb = h2 * 64
                                nc.tensor.matmul(
                                    ps[h2][:, j * 512 + t0 : (j + 1) * 512],
                                    lhsT=kt_sb[p][
                                        hb : hb + 64, ki * 128 : (ki + 1) * 128
                                    ],
                                    rhs=qt_sb[p][hb : hb + 64, q0 + t0 : q0 + 512],
                                    start=True,
                                    stop=True,
                                )
                        # exp from the first live column onward (cols below
                        # are never read by the ctx matmuls)
                        e0 = max(0, 128 * (2 * gi - 4 * qc))
                        for h2 in range(2):
                            nc.scalar.activation(
                                ex[h2][:, e0:1024],
                                ps[h2][:, e0:1024],
                                mybir.ActivationFunctionType.Exp,
                                scale=float(SCALE),
                            )
                        for j in range(2):
                            ki = 2 * gi + j
                            jj = ki - 4 * qc
                            if jj >= 0:  # triangular block on the diagonal
                                blk = slice(
                                    j * 512 + 128 * jj, j * 512 + 128 * jj + 128
                                )
                                for h2 in range(2):
                                    nc.vector.tensor_mul(
                                        ex[h2][:, blk], ex[h2][:, blk], tri_sb[:]
                                    )
                        if pend is not None:
                            pend()

                        def pend(gi=gi, ex=ex):
                            for j in range(2):
                                ki = 2 * gi + j
                                jj = ki - 4 * qc
                                t0 = max(0, 128 * jj)
                                for h2 in range(2):
                                    h = 2 * p + h2
                                    nc.tensor.matmul(
                                        ctx_ps[:, h2 * 512 + t0 : (h2 + 1) * 512],
                                        lhsT=vaug_ap(ki, h),
                                        rhs=ex[h2][:, j * 512 + t0 : (j + 1) * 512],
                                        start=(ki == 0),
                                        stop=(ki == nk - 1),
                                    )

                        slot += 1
                        for i in sched[slot]:
                            units[i]()
                    pend()
                    rec = rpool.tile(
                        [64, 1024], f32, name=f"rec{p}_{qc}{sfx}", tag="rec"
                    )
                    nc.vector.reciprocal(rec[:], ctx_ps[64:128, :])
                    for h2 in range(2):
                        nc.vector.tensor_mul(
                            ctxt_sb[p][h2 * 64 : h2 * 64 + 64, q0 : q0 + 512],
                            ctx_ps[0:64, h2 * 512 : (h2 + 1) * 512],
                            rec[:, h2 * 512 : (h2 + 1) * 512],
                        )

            # tail: the last chunk's out-projection, alternating the freed
            # score ring with aux for a deeper pipeline
            for i, nt in enumerate(range(N_SEQ // 128 - 4, N_SEQ // 128)):
                if i % 2 == 0:
                    unit_out(nt, pssc_pool, "sc")()
                else:
                    unit_out(nt, psaux_pool, "aux")()


def _build_module(reps=1):
    import concourse.bacc as bacc
    import concourse.mybir as mybir
    import concourse.tile as tile

    f32 = mybir.dt.float32
    bf16 = mybir.dt.bfloat16

    nc = bacc.Bacc()
    xT_d = nc.dram_tensor("xT", [D_EMB, N_SEQ], bf16, kind="ExternalInput")
    wqkv_d = nc.dram_tensor("wqkv", [D_EMB, 1536], bf16, kind="ExternalInput")
    wo_d = nc.dram_tensor("wo", [512, D_EMB], bf16, kind="ExternalInput")
    out_d = nc.dram_tensor("out", [N_SEQ, D_EMB], f32, kind="ExternalOutput")

    with tile.TileContext(nc) as tc:
        for rep in range(reps):
            _emit_body(
                nc, tc, mybir, f"_r{rep}" if reps > 1 else "",
                xT_d, wqkv_d, wo_d, out_d,
            )

    if not nc.is_finalized():
        nc.finalize()
    return nc


def _get_module(reps=1):
    key = f"nc{reps}"
    if key not in _CACHE:
        _CACHE[key] = _build_module(reps)
    return _CACHE[key]


def make_in_maps(x, W_q, W_k, W_v, W_o):
    in_maps = []
    for c in range(8):
        b, g = c // 2, c % 2
        gs = slice(g * 512, (g + 1) * 512)
        xT = np.ascontiguousarray(x[b].T).astype(BF16)
        wqkv = np.concatenate(
            [W_q[:, gs], W_k[:, gs], W_v[:, gs]], axis=1
        ).astype(BF16)
        wo = np.ascontiguousarray(W_o[gs, :]).astype(BF16)
        in_maps.append({"xT": xT, "wqkv": wqkv, "wo": wo})
    return in_maps


def kernel(x, W_q, W_k, W_v, W_o, b_o):
    from concourse.bass_utils import run_bass_kernel_spmd

    nc = _get_module()
    in_maps = make_in_maps(x, W_q, W_k, W_v, W_o)
    res = run_bass_kernel_spmd(nc, in_maps, core_ids=list(range(8)))

    out = np.empty((4, N_SEQ, D_EMB), np.float32)
    for b in range(4):
        out[b] = (
            res.results[2 * b]["out"]
            + res.results[2 * b + 1]["out"]
            + b_o[None, :].astype(np.float32)
        )
    return out


# revision 22
# speedup vs baseline: 1.0185x; 1.0022x over previous
"""Causal self-attention Trainium2 kernel (8 NeuronCores).

Sharding (Megatron-style, per sharding_hint):
  core c -> batch b = c//2, head-group g = c%2 (8 of 16 heads).
  W_q/W_k/W_v column-sliced per head group; W_o row-sliced; host sums the
  two partial outputs per batch (tensor-parallel reduce) and adds b_o.

Per-core kernel (all matmuls bf16 with fp32 PSUM accumulation):
  xT    [1024, 2048]  x[b] transposed (d_emb on partitions)
  wqkv  [1024, 1536]  [Wq_g | Wk_g | Wv_g]
  wo    [512, 1024]   W_o rows for this head group
  out   [2048, 1024]  fp32 partial (no bias)

Layouts: qT/kT stored [head_dim, n] so score matmuls contract over the
64-dim head axis; the two heads of a partition-tile occupy partitions
0:64 / 64:128, and their score matmuls are emitted interleaved so the PE
runs them concurrently in different row groups. Scores are computed
TRANSPOSED ([k, q]) so the exp'd weights feed the ctx matmul directly as
the moving operand; v is kept [n, head_dim] with a ones-block per head so
a single matmul yields both ctx^T and the softmax denominators broadcast
across 64 partitions.

Pipelining: QKV projection work is split into per-chunk units (v for the
chunk's 4 n-tiles, q/k for the chunk's 512 q-columns) and the units for
chunk qc+1 are emitted interleaved INSIDE attention chunk qc, so the PE
fills exp-latency gaps with projection matmuls and the scalar engine
starts exp'ing ~70us earlier than a proj-then-attend ordering. All PSUM
score/projection/out-proj tiles share one 2-slot pool ([128,1024], 4
banks) and the two ctx accumulators use the other 4 banks.

Causal handling: k-tiles strictly above the diagonal are skipped; on
diagonal sub-tiles the fully-masked query prefix is never computed
(scores and ctx matmuls trim their moving operand to q >= k-block start,
and exp starts at the first live column), and only the 128x128 diagonal
block gets a triangular bf16 multiply.

`reps` repeats the whole body inside one NEFF — used only for timing
((T(n)-T(1))/(n-1) cancels dispatch overhead); the graded path is reps=1.
"""

import sys

import numpy as np

sys.path.insert(0, "/opt/trn_rl_repo")

import ml_dtypes

BF16 = ml_dtypes.bfloat16

D_EMB = 1024
N_SEQ = 2048
N_HEADS_CORE = 8  # heads per core
HD = 64  # head dim
KT = D_EMB // 128  # 8 k-tiles over d_emb
PT = 4  # partition tiles over the 512 per-core head dims
NT = N_SEQ // 128  # 16 n-tiles
QC = N_SEQ // 512  # 4 query chunks of 512
SCALE = 1.0 / np.sqrt(HD)

_CACHE = {}


def _emit_body(nc, tc, mybir, sfx, xT_d, wqkv_d, wo_d, out_d):
    f32 = mybir.dt.float32
    bf16 = mybir.dt.bfloat16

    with tc.tile_pool(name=f"persist{sfx}", bufs=1) as persist:
        wo_sb = [
            persist.tile([128, D_EMB], bf16, name=f"wo{p}{sfx}", tag=f"wo{p}")
            for p in range(PT)
        ]
        qt_sb = [
            persist.tile([128, N_SEQ], bf16, name=f"qt{p}{sfx}", tag=f"qt{p}")
            for p in range(PT)
        ]
        kt_sb = [
            persist.tile([128, N_SEQ], bf16, name=f"kt{p}{sfx}", tag=f"kt{p}")
            for p in range(PT)
        ]
        ctxt_sb = [
            persist.tile([128, N_SEQ], bf16, name=f"ctxt{p}{sfx}", tag=f"ctxt{p}")
            for p in range(PT)
        ]
        # v per n-tile [128, 1024]: head h -> cols h*128:h*128+64 = v_h,
        # cols h*128+64:h*128+128 = 1.0 (softmax denominator ones-trick)
        v_sb = [
            persist.tile([128, 1024], bf16, name=f"v{nt}{sfx}", tag=f"v{nt}")
            for nt in range(NT)
        ]
        tri_sb = persist.tile([128, 128], bf16, name=f"tri{sfx}", tag="tri")
        xt_sb = [
            persist.tile([128, N_SEQ], bf16, name=f"xt{k}{sfx}", tag=f"xt{k}")
            for k in range(KT)
        ]
        wqkv_sb = [
            persist.tile([128, 1536], bf16, name=f"wqkv{k}{sfx}", tag=f"wqkv{k}")
            for k in range(KT)
        ]

        def vaug_ap(nt, h):
            return v_sb[nt][:, h * 128 : (h + 1) * 128]

        # ---- constants (gpsimd, no deps) ----
        for nt in range(NT):
            ones_view = v_sb[nt].rearrange("p (h c) -> p h c", h=N_HEADS_CORE)
            nc.gpsimd.memset(ones_view[:, :, 64:128], 1.0)
        # tri[k_local, q_local] = 1.0 if q_local >= k_local else 0
        nc.gpsimd.memset(tri_sb[:], 1.0)
        nc.gpsimd.affine_select(
            out=tri_sb[:],
            in_=tri_sb[:],
            compare_op=mybir.AluOpType.is_ge,
            fill=0.0,
            base=0,
            pattern=[[1, 128]],
            channel_multiplier=-1,
        )

        # ---- input DMA, balanced across the three DMA-capable queues ----
        for k in range(KT):
            xq = nc.sync if k % 2 == 0 else nc.scalar
            xq.dma_start(
                out=xt_sb[k][:], in_=xT_d[k * 128 : (k + 1) * 128, :]
            )
            wq = nc.gpsimd if k < 6 else nc.scalar
            wq.dma_start(
                out=wqkv_sb[k][:], in_=wqkv_d[k * 128 : (k + 1) * 128, :]
            )
        for p in range(PT):
            # wo is first needed by out(0) units draining in chunk 1 (~60us)
            nc.sync.dma_start(
                out=wo_sb[p][:], in_=wo_d[p * 128 : (p + 1) * 128, :]
            )

        with (
            tc.tile_pool(name=f"expp{sfx}", bufs=4) as expp,
            tc.tile_pool(name=f"rpool{sfx}", bufs=2) as rpool,
            tc.tile_pool(name=f"outp{sfx}", bufs=3) as outp,
            tc.tile_pool(name=f"pssc{sfx}", bufs=2, space="PSUM") as pssc_pool,
            tc.tile_pool(name=f"psctx{sfx}", bufs=1, space="PSUM") as psctx_pool,
            tc.tile_pool(name=f"psaux{sfx}", bufs=2, space="PSUM") as psaux_pool,
        ):
            # background work units, drained interleaved inside the
            # attention group loop. Drained units use the 2-slot 1-bank aux
            # PSUM ring so they never block on the score ring; fill-phase
            # units also borrow the score ring, idle before attention.
            def unit_v(nt, pool, tag):
                def emit():
                    ps = pool.tile(
                        [128, 512], f32, name=f"psv{nt}{sfx}", tag=tag
                    )
                    for k in range(KT):
                        nc.tensor.matmul(
                            ps[:],
                            lhsT=xt_sb[k][:, nt * 128 : (nt + 1) * 128],
                            rhs=wqkv_sb[k][:, 1024:1536],
                            start=(k == 0),
                            stop=(k == KT - 1),
                        )
                    v_view = v_sb[nt].rearrange("p (h c) -> p h c", h=N_HEADS_CORE)
                    nc.vector.tensor_copy(
                        v_view[:, :, 0:64],
                        ps.rearrange("p (h c) -> p h c", h=N_HEADS_CORE),
                    )

                return emit

            def unit_qk(which, p, qn, pool, tag):
                # which: 0 -> q, 1 -> k, for chunk qn, partition-tile p
                def emit():
                    nsl = slice(qn * 512, (qn + 1) * 512)
                    dst = (qt_sb, kt_sb)[which]
                    ps = pool.tile(
                        [128, 512], f32, name=f"psqk{which}_{p}_{qn}{sfx}",
                        tag=tag,
                    )
                    col0 = which * 512 + p * 128
                    for k in range(KT):
                        nc.tensor.matmul(
                            ps[:],
                            lhsT=wqkv_sb[k][:, col0 : col0 + 128],
                            rhs=xt_sb[k][:, nsl],
                            start=(k == 0),
                            stop=(k == KT - 1),
                        )
                    nc.vector.tensor_copy(dst[p][:, nsl], ps[:])

                return emit

            def unit_out(nt, pool, tag):
                # out = ctx @ Wo for one n-tile, in two 512-wide halves so
                # each copy+DMA drains while the other half accumulates
                def emit():
                    osb = outp.tile(
                        [128, 1024], f32, name=f"osb{nt}{sfx}", tag="osb"
                    )
                    for dh in range(2):
                        pso = pool.tile(
                            [128, 512], f32, name=f"pso{nt}_{dh}{sfx}",
                            tag=tag,
                        )
                        for p in range(PT):
                            nc.tensor.matmul(
                                pso[:],
                                lhsT=ctxt_sb[p][:, nt * 128 : (nt + 1) * 128],
                                rhs=wo_sb[p][:, dh * 512 : (dh + 1) * 512],
                                start=(p == 0),
                                stop=(p == PT - 1),
                            )
                        nc.vector.tensor_copy(
                            osb[:, dh * 512 : (dh + 1) * 512], pso[:]
                        )
                        nc.sync.dma_start(
                            out=out_d[
                                nt * 128 : (nt + 1) * 128,
                                dh * 512 : (dh + 1) * 512,
                            ],
                            in_=osb[:, dh * 512 : (dh + 1) * 512],
                        )

                return emit

            def vq(nt):
                return unit_v(nt, psaux_pool, "aux")

            def qkq(qn):
                us = []
                for p in range(PT):
                    us.append(unit_qk(0, p, qn, psaux_pool, "aux"))
                    us.append(unit_qk(1, p, qn, psaux_pool, "aux"))
                return us

            # fill phase: chunk 0's q/k/v, on the idle score+aux rings
            fill = []
            for p in range(PT):
                fill.append(unit_qk(0, p, 0, pssc_pool, "sc"))
                fill.append(unit_qk(1, p, 0, psaux_pool, "aux"))
            fill += [unit_v(nt, pssc_pool, "sc") for nt in range(4)]
            for u in fill:
                u()

            # per-chunk drain lists (deps: chunk qc's q/k/v units must drain
            # by chunk qc-1's end; out(c) drains any time after chunk c)
            drain = {
                0: qkq(1) + [vq(nt) for nt in range(4, 8)],
                1: qkq(2) + [vq(nt) for nt in range(8, 12)]
                + [unit_out(nt, psaux_pool, "aux") for nt in range(0, 4)],
                2: qkq(3) + [vq(nt) for nt in range(12, 16)]
                + [unit_out(nt, psaux_pool, "aux") for nt in range(4, 8)],
                3: [unit_out(nt, psaux_pool, "aux") for nt in range(8, 12)],
            }

            for qc in range(QC):
                q0 = qc * 512
                nk = 4 * qc + 4  # causal: k-tiles 0..nk-1
                ngroups = nk // 2
                units = drain[qc]
                nslots = PT * ngroups
                sched = [[] for _ in range(nslots + 1)]
                for i in range(len(units)):
                    sched[round((i + 1) * nslots / len(units))].append(i)
                slot = 0
                for p in range(PT):
                    # both heads' ctx in one 2-bank tile: h2 -> cols h2*512
                    ctx_ps = psctx_pool.tile(
                        [128, 1024], f32, name=f"ctx{p}_{qc}{sfx}", tag="ctx"
                    )
                    pend = None  # deferred ctx-matmul emission (1-group lag)
                    for gi in range(ngroups):
                        ps = [
                            pssc_pool.tile(
                                [128, 1024],
                                f32,
                                name=f"sc{p}_{qc}_{gi}_{h2}{sfx}",
                                tag="sc",
                            )
                            for h2 in range(2)
                        ]
                        ex = [
                            expp.tile(
                                [128, 1024],
                                bf16,
                                name=f"ex{p}_{qc}_{gi}_{h2}{sfx}",
                                tag="ex",
                            )
                            for h2 in range(2)
                        ]
                        # interleave heads so PE overlaps the row-group pairs
                        for j in range(2):
                            ki = 2 * gi + j
                            jj = ki - 4 * qc  # >=0 on diagonal sub-tiles
                            t0 = max(0, 128 * jj)  # masked-prefix trim
                            for h2 in range(2):
                                hb = h2 * 64
                                nc.tensor.matmul(
                                    ps[h2][:, j * 512 + t0 : (j + 1) * 512],
                                    lhsT=kt_sb[p][
                                        hb : hb + 64, ki * 128 : (ki + 1) * 128
                                    ],
                                    rhs=qt_sb[p][hb : hb + 64, q0 + t0 : q0 + 512],
                                    start=True,
                                    stop=True,
                                )
                        # exp only live columns: on deep-diagonal groups the
                        # two j-halves are exp'd separately to skip the
                        # masked prefix of each (worth it when the j=1
                        # prefix exceeds the ~228-cycle instruction cost)
                        jj0 = 2 * gi - 4 * qc
                        if jj0 >= 1:
                            spans = [
                                (128 * jj0, 512),
                                (512 + 128 * (jj0 + 1), 1024),
                            ]
                        else:
                            spans = [(max(0, 128 * jj0), 1024)]
                        for h2 in range(2):
                            for s0, s1 in spans:
                                nc.scalar.activation(
                                    ex[h2][:, s0:s1],
                                    ps[h2][:, s0:s1],
                                    mybir.ActivationFunctionType.Exp,
                                    scale=float(SCALE),
                                )
                        for j in range(2):
                            ki = 2 * gi + j
                            jj = ki - 4 * qc
                            if jj >= 0:  # triangular block on the diagonal
                                blk = slice(
                                    j * 512 + 128 * jj, j * 512 + 128 * jj + 128
                                )
                                for h2 in range(2):
                                    nc.vector.tensor_mul(
                                        ex[h2][:, blk], ex[h2][:, blk], tri_sb[:]
                                    )
                        if pend is not None:
                            pend()

                        def pend(gi=gi, ex=ex):
                            for j in range(2):
                                ki = 2 * gi + j
                                jj = ki - 4 * qc
                                t0 = max(0, 128 * jj)
                                for h2 in range(2):
                                    h = 2 * p + h2
                                    nc.tensor.matmul(
                                        ctx_ps[:, h2 * 512 + t0 : (h2 + 1) * 512],
                                        lhsT=vaug_ap(ki, h),
                                        rhs=ex[h2][:, j * 512 + t0 : (j + 1) * 512],
                                        start=(ki == 0),
                                        stop=(ki == nk - 1),
                                    )

                        slot += 1
                        for i in sched[slot]:
                            units[i]()
                    pend()
                    rec = rpool.tile(
                        [64, 1024], f32, name=f"rec{p}_{qc}{sfx}", tag="rec"
                    )
                    nc.vector.reciprocal(rec[:], ctx_ps[64:128, :])
                    for h2 in range(2):
                        nc.vector.tensor_mul(
                            ctxt_sb[p][h2 * 64 : h2 * 64 + 64, q0 : q0 + 512],
                            ctx_ps[0:64, h2 * 512 : (h2 + 1) * 512],
                            rec[:, h2 * 512 : (h2 + 1) * 512],
                        )

            # tail: the last chunk's out-projection, alternating the freed
            # score ring with aux for a deeper pipeline
            for i, nt in enumerate(range(N_SEQ // 128 - 4, N_SEQ // 128)):
                if i % 2 == 0:
                    unit_out(nt, pssc_pool, "sc")()
                else:
                    unit_out(nt, psaux_pool, "aux")()


def _build_module(reps=1):
    import concourse.bacc as bacc
    import concourse.mybir as mybir
    import concourse.tile as tile

    f32 = mybir.dt.float32
    bf16 = mybir.dt.bfloat16

    nc = bacc.Bacc()
    xT_d = nc.dram_tensor("xT", [D_EMB, N_SEQ], bf16, kind="ExternalInput")
    wqkv_d = nc.dram_tensor("wqkv", [D_EMB, 1536], bf16, kind="ExternalInput")
    wo_d = nc.dram_tensor("wo", [512, D_EMB], bf16, kind="ExternalInput")
    out_d = nc.dram_tensor("out", [N_SEQ, D_EMB], f32, kind="ExternalOutput")

    with tile.TileContext(nc) as tc:
        for rep in range(reps):
            _emit_body(
                nc, tc, mybir, f"_r{rep}" if reps > 1 else "",
                xT_d, wqkv_d, wo_d, out_d,
            )

    if not nc.is_finalized():
        nc.finalize()
    return nc


def _get_module(reps=1):
    key = f"nc{reps}"
    if key not in _CACHE:
        _CACHE[key] = _build_module(reps)
    return _CACHE[key]


def make_in_maps(x, W_q, W_k, W_v, W_o):
    in_maps = []
    for c in range(8):
        b, g = c // 2, c % 2
        gs = slice(g * 512, (g + 1) * 512)
        xT = np.ascontiguousarray(x[b].T).astype(BF16)
        wqkv = np.concatenate(
            [W_q[:, gs], W_k[:, gs], W_v[:, gs]], axis=1
        ).astype(BF16)
        wo = np.ascontiguousarray(W_o[gs, :]).astype(BF16)
        in_maps.append({"xT": xT, "wqkv": wqkv, "wo": wo})
    return in_maps


def kernel(x, W_q, W_k, W_v, W_o, b_o):
    from concourse.bass_utils import run_bass_kernel_spmd

    nc = _get_module()
    in_maps = make_in_maps(x, W_q, W_k, W_v, W_o)
    res = run_bass_kernel_spmd(nc, in_maps, core_ids=list(range(8)))

    out = np.empty((4, N_SEQ, D_EMB), np.float32)
    for b in range(4):
        out[b] = (
            res.results[2 * b]["out"]
            + res.results[2 * b + 1]["out"]
            + b_o[None, :].astype(np.float32)
        )
    return out


# revision 24
# speedup vs baseline: 1.0247x; 1.0061x over previous
"""Causal self-attention Trainium2 kernel (8 NeuronCores).

Sharding (Megatron-style, per sharding_hint):
  core c -> batch b = c//2, head-group g = c%2 (8 of 16 heads).
  W_q/W_k/W_v column-sliced per head group; W_o row-sliced; host sums the
  two partial outputs per batch (tensor-parallel reduce) and adds b_o.

Per-core kernel (all matmuls bf16 with fp32 PSUM accumulation):
  xT    [1024, 2048]  x[b] transposed (d_emb on partitions)
  wqkv  [1024, 1536]  [Wq_g | Wk_g | Wv_g]
  wo    [512, 1024]   W_o rows for this head group
  out   [2048, 1024]  fp32 partial (no bias)

Layouts: qT/kT stored [head_dim, n] so score matmuls contract over the
64-dim head axis; the two heads of a partition-tile occupy partitions
0:64 / 64:128, and their score matmuls are emitted interleaved so the PE
runs them concurrently in different row groups. Scores are computed
TRANSPOSED ([k, q]) so the exp'd weights feed the ctx matmul directly as
the moving operand; v is kept [n, head_dim] with a ones-block per head so
a single matmul yields both ctx^T and the softmax denominators broadcast
across 64 partitions.

Pipelining: QKV projection work is split into per-chunk units (v for the
chunk's 4 n-tiles, q/k for the chunk's 512 q-columns) and the units for
chunk qc+1 are emitted interleaved INSIDE attention chunk qc, so the PE
fills exp-latency gaps with projection matmuls and the scalar engine
starts exp'ing ~70us earlier than a proj-then-attend ordering. All PSUM
score/projection/out-proj tiles share one 2-slot pool ([128,1024], 4
banks) and the two ctx accumulators use the other 4 banks.

Causal handling: k-tiles strictly above the diagonal are skipped; on
diagonal sub-tiles the fully-masked query prefix is never computed
(scores and ctx matmuls trim their moving operand to q >= k-block start,
and exp starts at the first live column), and only the 128x128 diagonal
block gets a triangular bf16 multiply.

`reps` repeats the whole body inside one NEFF — used only for timing
((T(n)-T(1))/(n-1) cancels dispatch overhead); the graded path is reps=1.
"""

import sys

import numpy as np

sys.path.insert(0, "/opt/trn_rl_repo")

import ml_dtypes

BF16 = ml_dtypes.bfloat16

D_EMB = 1024
N_SEQ = 2048
N_HEADS_CORE = 8  # heads per core
HD = 64  # head dim
KT = D_EMB // 128  # 8 k-tiles over d_emb
PT = 4  # partition tiles over the 512 per-core head dims
NT = N_SEQ // 128  # 16 n-tiles
QC = N_SEQ // 512  # 4 query chunks of 512
SCALE = 1.0 / np.sqrt(HD)

_CACHE = {}


def _emit_body(nc, tc, mybir, sfx, xT_d, wqkv_d, wo_d, out_d):
    f32 = mybir.dt.float32
    bf16 = mybir.dt.bfloat16

    with tc.tile_pool(name=f"persist{sfx}", bufs=1) as persist:
        wo_sb = [
            persist.tile([128, D_EMB], bf16, name=f"wo{p}{sfx}", tag=f"wo{p}")
            for p in range(PT)
        ]
        qt_sb = [
            persist.tile([128, N_SEQ], bf16, name=f"qt{p}{sfx}", tag=f"qt{p}")
            for p in range(PT)
        ]
        kt_sb = [
            persist.tile([128, N_SEQ], bf16, name=f"kt{p}{sfx}", tag=f"kt{p}")
            for p in range(PT)
        ]
        ctxt_sb = [
            persist.tile([128, N_SEQ], bf16, name=f"ctxt{p}{sfx}", tag=f"ctxt{p}")
            for p in range(PT)
        ]
        # v per n-tile [128, 1024]: head h -> cols h*128:h*128+64 = v_h,
        # cols h*128+64:h*128+128 = 1.0 (softmax denominator ones-trick)
        v_sb = [
            persist.tile([128, 1024], bf16, name=f"v{nt}{sfx}", tag=f"v{nt}")
            for nt in range(NT)
        ]
        tri_sb = persist.tile([128, 128], bf16, name=f"tri{sfx}", tag="tri")
        xt_sb = [
            persist.tile([128, N_SEQ], bf16, name=f"xt{k}{sfx}", tag=f"xt{k}")
            for k in range(KT)
        ]
        wqkv_sb = [
            persist.tile([128, 1536], bf16, name=f"wqkv{k}{sfx}", tag=f"wqkv{k}")
            for k in range(KT)
        ]

        def vaug_ap(nt, h):
            return v_sb[nt][:, h * 128 : (h + 1) * 128]

        # ---- constants (gpsimd, no deps) ----
        for nt in range(NT):
            ones_view = v_sb[nt].rearrange("p (h c) -> p h c", h=N_HEADS_CORE)
            nc.gpsimd.memset(ones_view[:, :, 64:128], 1.0)
        # tri[k_local, q_local] = 1.0 if q_local >= k_local else 0
        nc.gpsimd.memset(tri_sb[:], 1.0)
        nc.gpsimd.affine_select(
            out=tri_sb[:],
            in_=tri_sb[:],
            compare_op=mybir.AluOpType.is_ge,
            fill=0.0,
            base=0,
            pattern=[[1, 128]],
            channel_multiplier=-1,
        )

        # ---- input DMA, balanced across the three DMA-capable queues ----
        for k in range(KT):
            xq = nc.sync if k % 2 == 0 else nc.scalar
            xq.dma_start(
                out=xt_sb[k][:], in_=xT_d[k * 128 : (k + 1) * 128, :]
            )
            wq = nc.gpsimd if k < 6 else nc.scalar
            wq.dma_start(
                out=wqkv_sb[k][:], in_=wqkv_d[k * 128 : (k + 1) * 128, :]
            )
        for p in range(PT):
            # wo is first needed by out(0) units draining in chunk 1 (~60us)
            nc.sync.dma_start(
                out=wo_sb[p][:], in_=wo_d[p * 128 : (p + 1) * 128, :]
            )

        with (
            tc.tile_pool(name=f"expp{sfx}", bufs=4) as expp,
            tc.tile_pool(name=f"rpool{sfx}", bufs=2) as rpool,
            tc.tile_pool(name=f"outp{sfx}", bufs=3) as outp,
            tc.tile_pool(name=f"pssc{sfx}", bufs=2, space="PSUM") as pssc_pool,
            tc.tile_pool(name=f"psctx{sfx}", bufs=1, space="PSUM") as psctx_pool,
            tc.tile_pool(name=f"psaux{sfx}", bufs=2, space="PSUM") as psaux_pool,
        ):
            # background work units, drained interleaved inside the
            # attention group loop. Drained units use the 2-slot 1-bank aux
            # PSUM ring so they never block on the score ring; fill-phase
            # units also borrow the score ring, idle before attention.
            def unit_v(nt, pool, tag):
                def emit():
                    ps = pool.tile(
                        [128, 512], f32, name=f"psv{nt}{sfx}", tag=tag
                    )
                    for k in range(KT):
                        nc.tensor.matmul(
                            ps[:],
                            lhsT=xt_sb[k][:, nt * 128 : (nt + 1) * 128],
                            rhs=wqkv_sb[k][:, 1024:1536],
                            start=(k == 0),
                            stop=(k == KT - 1),
                        )
                    v_view = v_sb[nt].rearrange("p (h c) -> p h c", h=N_HEADS_CORE)
                    nc.vector.tensor_copy(
                        v_view[:, :, 0:64],
                        ps.rearrange("p (h c) -> p h c", h=N_HEADS_CORE),
                    )

                return emit

            def unit_qk(which, p, qn, pool, tag):
                # which: 0 -> q, 1 -> k, for chunk qn, partition-tile p
                def emit():
                    nsl = slice(qn * 512, (qn + 1) * 512)
                    dst = (qt_sb, kt_sb)[which]
                    ps = pool.tile(
                        [128, 512], f32, name=f"psqk{which}_{p}_{qn}{sfx}",
                        tag=tag,
                    )
                    col0 = which * 512 + p * 128
                    for k in range(KT):
                        nc.tensor.matmul(
                            ps[:],
                            lhsT=wqkv_sb[k][:, col0 : col0 + 128],
                            rhs=xt_sb[k][:, nsl],
                            start=(k == 0),
                            stop=(k == KT - 1),
                        )
                    nc.vector.tensor_copy(dst[p][:, nsl], ps[:])

                return emit

            def unit_out(nt, pool, tag):
                # out = ctx @ Wo for one n-tile, in two 512-wide halves so
                # each copy+DMA drains while the other half accumulates
                def emit():
                    osb = outp.tile(
                        [128, 1024], f32, name=f"osb{nt}{sfx}", tag="osb"
                    )
                    for dh in range(2):
                        pso = pool.tile(
                            [128, 512], f32, name=f"pso{nt}_{dh}{sfx}",
                            tag=tag,
                        )
                        for p in range(PT):
                            nc.tensor.matmul(
                                pso[:],
                                lhsT=ctxt_sb[p][:, nt * 128 : (nt + 1) * 128],
                                rhs=wo_sb[p][:, dh * 512 : (dh + 1) * 512],
                                start=(p == 0),
                                stop=(p == PT - 1),
                            )
                        nc.vector.tensor_copy(
                            osb[:, dh * 512 : (dh + 1) * 512], pso[:]
                        )
                        nc.sync.dma_start(
                            out=out_d[
                                nt * 128 : (nt + 1) * 128,
                                dh * 512 : (dh + 1) * 512,
                            ],
                            in_=osb[:, dh * 512 : (dh + 1) * 512],
                        )

                return emit

            def vq(nt):
                return unit_v(nt, psaux_pool, "aux")

            def qkq(qn):
                us = []
                for p in range(PT):
                    us.append(unit_qk(0, p, qn, psaux_pool, "aux"))
                    us.append(unit_qk(1, p, qn, psaux_pool, "aux"))
                return us

            # fill phase: chunk 0's q/k/v, on the idle score+aux rings
            fill = []
            for p in range(PT):
                fill.append(unit_qk(0, p, 0, pssc_pool, "sc"))
                fill.append(unit_qk(1, p, 0, psaux_pool, "aux"))
            fill += [unit_v(nt, pssc_pool, "sc") for nt in range(4)]
            for u in fill:
                u()

            # per-chunk drain lists (deps: chunk qc's q/k/v units must drain
            # by chunk qc-1's end; out(c) drains any time after chunk c)
            drain = {
                0: qkq(1) + [vq(nt) for nt in range(4, 8)],
                1: qkq(2) + [vq(nt) for nt in range(8, 12)]
                + [unit_out(nt, psaux_pool, "aux") for nt in range(0, 4)],
                2: qkq(3)
                + [unit_out(nt, psaux_pool, "aux") for nt in range(4, 8)],
                3: [vq(nt) for nt in range(12, 16)]
                + [unit_out(nt, psaux_pool, "aux") for nt in range(8, 12)],
            }

            for qc in range(QC):
                q0 = qc * 512
                nk = 4 * qc + 4  # causal: k-tiles 0..nk-1
                ngroups = nk // 2
                units = drain[qc]
                nslots = PT * ngroups
                sched = [[] for _ in range(nslots + 1)]
                # chunk 3's own v units (first 4 of its drain list) are read
                # by its gi>=6 groups (slots 7-8): front-load them
                front = 4 if qc == QC - 1 else 0
                for i in range(front):
                    sched[i + 1].append(i)
                rest = len(units) - front
                for i in range(front, len(units)):
                    sched[
                        front
                        + round((i - front + 1) * (nslots - front) / rest)
                    ].append(i)
                slot = 0
                for p in range(PT):
                    # both heads' ctx in one 2-bank tile: h2 -> cols h2*512
                    ctx_ps = psctx_pool.tile(
                        [128, 1024], f32, name=f"ctx{p}_{qc}{sfx}", tag="ctx"
                    )
                    pend = None  # deferred ctx-matmul emission (1-group lag)
                    for gi in range(ngroups):
                        ps = [
                            pssc_pool.tile(
                                [128, 1024],
                                f32,
                                name=f"sc{p}_{qc}_{gi}_{h2}{sfx}",
                                tag="sc",
                            )
                            for h2 in range(2)
                        ]
                        ex = [
                            expp.tile(
                                [128, 1024],
                                bf16,
                                name=f"ex{p}_{qc}_{gi}_{h2}{sfx}",
                                tag="ex",
                            )
                            for h2 in range(2)
                        ]
                        # interleave heads so PE overlaps the row-group pairs
                        for j in range(2):
                            ki = 2 * gi + j
                            jj = ki - 4 * qc  # >=0 on diagonal sub-tiles
                            t0 = max(0, 128 * jj)  # masked-prefix trim
                            for h2 in range(2):
                                hb = h2 * 64
                                nc.tensor.matmul(
                                    ps[h2][:, j * 512 + t0 : (j + 1) * 512],
                                    lhsT=kt_sb[p][
                                        hb : hb + 64, ki * 128 : (ki + 1) * 128
                                    ],
                                    rhs=qt_sb[p][hb : hb + 64, q0 + t0 : q0 + 512],
                                    start=True,
                                    stop=True,
                                )
                        # exp only live columns: on deep-diagonal groups the
                        # two j-halves are exp'd separately to skip the
                        # masked prefix of each (worth it when the j=1
                        # prefix exceeds the ~228-cycle instruction cost)
                        jj0 = 2 * gi - 4 * qc
                        if jj0 >= 1:
                            spans = [
                                (128 * jj0, 512),
                                (512 + 128 * (jj0 + 1), 1024),
                            ]
                        else:
                            spans = [(max(0, 128 * jj0), 1024)]
                        for h2 in range(2):
                            for s0, s1 in spans:
                                nc.scalar.activation(
                                    ex[h2][:, s0:s1],
                                    ps[h2][:, s0:s1],
                                    mybir.ActivationFunctionType.Exp,
                                    scale=float(SCALE),
                                )
                        for j in range(2):
                            ki = 2 * gi + j
                            jj = ki - 4 * qc
                            if jj >= 0:  # triangular block on the diagonal
                                blk = slice(
                                    j * 512 + 128 * jj, j * 512 + 128 * jj + 128
                                )
                                for h2 in range(2):
                                    nc.vector.tensor_mul(
                                        ex[h2][:, blk], ex[h2][:, blk], tri_sb[:]
                                    )
                        if pend is not None:
                            pend()

                        def pend(gi=gi, ex=ex):
                            for j in range(2):
                                ki = 2 * gi + j
                                jj = ki - 4 * qc
                                t0 = max(0, 128 * jj)
                                for h2 in range(2):
                                    h = 2 * p + h2
                                    nc.tensor.matmul(
                                        ctx_ps[:, h2 * 512 + t0 : (h2 + 1) * 512],
                                        lhsT=vaug_ap(ki, h),
                                        rhs=ex[h2][:, j * 512 + t0 : (j + 1) * 512],
                                        start=(ki == 0),
                                        stop=(ki == nk - 1),
                                    )

                        slot += 1
                        for i in sched[slot]:
                            units[i]()
                    pend()
                    rec = rpool.tile(
                        [64, 1024], f32, name=f"rec{p}_{qc}{sfx}", tag="rec"
                    )
                    nc.vector.reciprocal(rec[:], ctx_ps[64:128, :])
                    for h2 in range(2):
                        nc.vector.tensor_mul(
                            ctxt_sb[p][h2 * 64 : h2 * 64 + 64, q0 : q0 + 512],
                            ctx_ps[0:64, h2 * 512 : (h2 + 1) * 512],
                            rec[:, h2 * 512 : (h2 + 1) * 512],
                        )

            # tail: the last chunk's out-projection, alternating the freed
            # score ring with aux for a deeper pipeline
            for i, nt in enumerate(range(N_SEQ // 128 - 4, N_SEQ // 128)):
                if i % 2 == 0:
                    unit_out(nt, pssc_pool, "sc")()
                else:
                    unit_out(nt, psaux_pool, "aux")()


def _build_module(reps=1):
    import concourse.bacc as bacc
    import concourse.mybir as mybir
    import concourse.tile as tile

    f32 = mybir.dt.float32
    bf16 = mybir.dt.bfloat16

    nc = bacc.Bacc()
    xT_d = nc.dram_tensor("xT", [D_EMB, N_SEQ], bf16, kind="ExternalInput")
    wqkv_d = nc.dram_tensor("wqkv", [D_EMB, 1536], bf16, kind="ExternalInput")
    wo_d = nc.dram_tensor("wo", [512, D_EMB], bf16, kind="ExternalInput")
    out_d = nc.dram_tensor("out", [N_SEQ, D_EMB], f32, kind="ExternalOutput")

    with tile.TileContext(nc) as tc:
        for rep in range(reps):
            _emit_body(
                nc, tc, mybir, f"_r{rep}" if reps > 1 else "",
                xT_d, wqkv_d, wo_d, out_d,
            )

    if not nc.is_finalized():
        nc.finalize()
    return nc


def _get_module(reps=1):
    key = f"nc{reps}"
    if key not in _CACHE:
        _CACHE[key] = _build_module(reps)
    return _CACHE[key]


def make_in_maps(x, W_q, W_k, W_v, W_o):
    in_maps = []
    for c in range(8):
        b, g = c // 2, c % 2
        gs = slice(g * 512, (g + 1) * 512)
        xT = np.ascontiguousarray(x[b].T).astype(BF16)
        wqkv = np.concatenate(
            [W_q[:, gs], W_k[:, gs], W_v[:, gs]], axis=1
        ).astype(BF16)
        wo = np.ascontiguousarray(W_o[gs, :]).astype(BF16)
        in_maps.append({"xT": xT, "wqkv": wqkv, "wo": wo})
    return in_maps


def kernel(x, W_q, W_k, W_v, W_o, b_o):
    from concourse.bass_utils import run_bass_kernel_spmd

    nc = _get_module()
    in_maps = make_in_maps(x, W_q, W_k, W_v, W_o)
    res = run_bass_kernel_spmd(nc, in_maps, core_ids=list(range(8)))

    out = np.empty((4, N_SEQ, D_EMB), np.float32)
    for b in range(4):
        out[b] = (
            res.results[2 * b]["out"]
            + res.results[2 * b + 1]["out"]
            + b_o[None, :].astype(np.float32)
        )
    return out
